# revision 52
# baseline (speedup 1.0000x reference)
"""MLA attention kernel for Trainium2 — 8-core tensor-parallel (self-contained).

Sharding: data-parallel over batch (2) x tensor-parallel over head groups
(4 groups of 4 heads) = 8 cores, SPMD (one NEFF, per-core input shards).
Core ci: batch ci//4, heads [4*(ci%4), 4*(ci%4)+4).

Per-core dataflow (everything feature-major "transposed" so the PE never
needs an on-chip transpose):
  x^T quarter + weights streamed in up front (wkvd/xt interleaved first)
  kv^T = wkvd.T @ x^T (rmsnorm sum via ones-matmul, scale broadcast via K=1
    matmul); normalized latents AllGather'd in bf16 (dispatched early so the
    wire hides under the q path)
  q_lat^T = wqd.T @ x^T, AllGather'd bf16 in two token halves so q-up can
    start on the first half while the second is still on the wire
  kv-up / q-up consume the gathered latents; k_nope^T / v / q^T / q_rope^T
    live in SBUF-RESIDENT tiles -- phase B issues ZERO DMA.
  scores^T[k,q] = k^T.T @ q^T -> exp (ACT, scale folded) -> causal mask (GpSimd)
  denom[1,q] = ones.T @ E ; out_h^T[v,q] = v.T @ E, drained in 4-kc blocks so
    same-PSUM-bank matmuls stay adjacent (short accumulation groups pay a
    ~150ns group-start penalty on the PE)
  normalize via K=1 broadcast matmul of 1/denom, deferred one iteration
  out[t, hid] = attn^T.T @ w_out (token-major, bf16 contiguous writes)
Host: sums the 4 bf16 partial outputs per batch in fp32.
"""

import math

import numpy as np
import ml_dtypes

# ---- problem constants (from the reference model) ----
B, S, HID = 2, 2048, 2048
H, D_NOPE, D_ROPE, V_DIM = 16, 128, 64, 128
KV_RANK, Q_RANK = 512, 1536
HEAD_DIM = D_NOPE + D_ROPE
THETA, EPS = 10000.0, 1e-6
NCORES = 8
NH = 4                    # heads per core
T = 512                   # phase-A token chunk
NT = S // T
QC = 512                  # attention query chunk
NQC = S // QC
KH = HID // 128           # 16 k-chunks over HID
RQ = Q_RANK // 128        # 12 chunks over q rank
RKV = KV_RANK // 128      # 4 chunks over kv rank
SCALE = 1.0 / math.sqrt(HEAD_DIM)

_CACHE = {}


def build_nc():
    """Build the Bass/Tile program (one NeuronCore, run SPMD on 8)."""
    from contextlib import ExitStack

    import concourse.mybir as mybir
    import concourse.tile as tile
    from concourse import bacc
    from concourse.bass import ds

    dt = mybir.dt
    AF = mybir.ActivationFunctionType
    bf16 = dt.bfloat16
    f32 = dt.float32

    nc = bacc.Bacc(
        "TRN2",
        target_bir_lowering=False,
        debug=False,
        enable_asserts=False,
        num_devices=NCORES,
    )

    # ---- I/O ----
    x_ap = nc.dram_tensor("x", [HID, S // 4], bf16, kind="ExternalInput").ap()
    wqd_ap = nc.dram_tensor("wqd", [HID, Q_RANK], bf16, kind="ExternalInput").ap()
    wqu_ap = nc.dram_tensor("wqu", [Q_RANK, NH * HEAD_DIM], bf16, kind="ExternalInput").ap()
    cos_ap = nc.dram_tensor("cosq", [128, S], bf16, kind="ExternalInput").ap()
    sin_ap = nc.dram_tensor("sinq", [128, S], bf16, kind="ExternalInput").ap()
    wkvd_ap = nc.dram_tensor("wkvd", [HID, KV_RANK + D_ROPE], bf16, kind="ExternalInput").ap()
    wkvuk_ap = nc.dram_tensor("wkvuk", [KV_RANK, NH * D_NOPE], bf16, kind="ExternalInput").ap()
    wkvuv_ap = nc.dram_tensor("wkvuv", [KV_RANK, NH * V_DIM], bf16, kind="ExternalInput").ap()
    wout_ap = nc.dram_tensor("wout", [NH * V_DIM, HID], bf16, kind="ExternalInput").ap()
    mask_ap = nc.dram_tensor("maskt", [128, 1024], bf16, kind="ExternalInput").ap()
    onesc_ap = nc.dram_tensor("ones_col", [128, 1], bf16, kind="ExternalInput").ap()
    onesr_ap = nc.dram_tensor("ones_row", [1, 128], f32, kind="ExternalInput").ap()
    cosl_ap = nc.dram_tensor("cosl", [128, S // 4], bf16, kind="ExternalInput").ap()
    sinl_ap = nc.dram_tensor("sinl", [128, S // 4], bf16, kind="ExternalInput").ap()
    out_ap = nc.dram_tensor("out", [S, HID], bf16, kind="ExternalOutput").ap()

    with tile.TileContext(nc) as tc, ExitStack() as ctx:
        const = ctx.enter_context(tc.tile_pool(name="const", bufs=1))
        dram = ctx.enter_context(tc.tile_pool(name="dram", bufs=1, space="DRAM"))
        mm_ps = ctx.enter_context(tc.tile_pool(name="mm_ps", bufs=4, space="PSUM"))
        pv_ps = ctx.enter_context(tc.tile_pool(name="pv_ps", bufs=2, space="PSUM"))
        sm_ps = ctx.enter_context(tc.tile_pool(name="sm_ps", bufs=1, space="PSUM"))
        bc_ps = ctx.enter_context(tc.tile_pool(name="bc_ps", bufs=1, space="PSUM"))

        TL = S // 4  # local token quarter
        TH = TL // 2  # gather token half

        # ---- pools by lifetime (releases must nest LIFO) ----
        wAp = tc.alloc_tile_pool(name="wAp", bufs=1)  # wqu slice (small, kept)
        w0 = tc.alloc_tile_pool(name="w0", bufs=1)    # wkvd, xt, wqd: dead after A0

        # DMA issue order is consumption order: wkvd/xt interleaved (kv down
        # paces with these), then wqd, then the q-up slice.
        wkvd_sb = w0.tile([128, KH, KV_RANK + D_ROPE], bf16, name="wkvd_sb")
        xt = w0.tile([128, KH, TL], bf16, name="xt_sb")
        for k in range(KH):
            nc.sync.dma_start(out=wkvd_sb[:, k, :], in_=wkvd_ap[ds(k * 128, 128), :])
            nc.sync.dma_start(out=xt[:, k, :], in_=x_ap[ds(k * 128, 128), :])
        wqd_sb = w0.tile([128, KH, Q_RANK], bf16, name="wqd_sb")
        for k in range(KH):
            nc.sync.dma_start(out=wqd_sb[:, k, :], in_=wqd_ap[ds(k * 128, 128), :])
        wqu_sb = wAp.tile([128, RQ, NH * HEAD_DIM], bf16, name="wqu_sb")
        for r in range(RQ):
            nc.sync.dma_start(out=wqu_sb[:, r, :], in_=wqu_ap[ds(r * 128, 128), :])

        # small constants on the gpsimd queue (doesn't delay the weight stream)
        cosl_sb = const.tile([128, TL], bf16, name="cosl_sb")
        nc.gpsimd.dma_start(out=cosl_sb[:], in_=cosl_ap[:])
        sinl_sb = const.tile([128, TL], bf16, name="sinl_sb")
        nc.gpsimd.dma_start(out=sinl_sb[:], in_=sinl_ap[:])
        onesc_sb = const.tile([128, 1], bf16, name="onesc_sb")
        nc.gpsimd.dma_start(out=onesc_sb[:], in_=onesc_ap[:])
        onesr_f32 = const.tile([1, 128], f32, name="onesr_f32")
        nc.gpsimd.dma_start(out=onesr_f32[:], in_=onesr_ap[:])
        onesr_sb = const.tile([1, 128], dt.float32r, name="onesr_sb")
        with nc.allow_low_precision(reason="exact ones rounded to f32r"):
            nc.vector.tensor_copy(onesr_sb[:], onesr_f32[:])
        eps_sb = const.tile([1, 1], f32, name="eps_sb")
        nc.gpsimd.memset(eps_sb[:], EPS)

        # DRAM collective bounce buffers (q latent gathered in token halves
        # so its wire pipelines into half-granular q-up work)
        # Payloads are partition-major [128, free] so each consumer quarter
        # unpacks with ONE wide DMA instead of one per rank-chunk.  k-rope is
        # folded into [128, TL/2] (token halves stacked) — zero wire waste.
        gin_kv = dram.tile([128, RKV * TL + TH], bf16, name="gin_kv")
        gout_kv = dram.tile([4, 128, RKV * TL + TH], bf16, name="gout_kv")
        gin_qa = dram.tile([128, RQ * TH], bf16, name="gin_qa")
        gout_qa = dram.tile([4, 128, RQ * TH], bf16, name="gout_qa")
        gin_qb = dram.tile([128, RQ * TH], bf16, name="gin_qb")
        gout_qb = dram.tile([4, 128, RQ * TH], bf16, name="gout_qb")
        GROUPS = [[0, 1, 2, 3], [4, 5, 6, 7]]

        # ================= phase A0: local down-projections =================
        workA0 = tc.alloc_tile_pool(name="workA0", bufs=2)
        work = workA0
        # ---- kv down (local quarter) ----
        kvc_bf = work.tile([128, RKV, TL], bf16, tag="kvc", bufs=1)
        sq_bf = work.tile([128, RKV, TL], bf16, tag="sq", bufs=1)
        for j in range(RKV):
            ps = mm_ps.tile([128, TL], f32, tag="mm")
            for k in range(KH):
                nc.tensor.matmul(
                    ps, wkvd_sb[:, k, ds(j * 128, 128)], xt[:, k, :],
                    start=(k == 0), stop=(k == KH - 1),
                )
            nc.scalar.activation(sq_bf[:, j, :], ps, AF.Square)
            nc.vector.tensor_copy(kvc_bf[:, j, :], ps)
        ms = sm_ps.tile([1, TL], f32, tag="rowps", bufs=1)
        for j in range(RKV):
            nc.tensor.matmul(
                ms, onesc_sb[:], sq_bf[:, j, :],
                start=(j == 0), stop=(j == RKV - 1),
            )
        krp = mm_ps.tile([64, TL], f32, tag="mm")
        for k in range(KH):
            nc.tensor.matmul(
                krp, wkvd_sb[:, k, ds(KV_RANK, D_ROPE)], xt[:, k, :],
                start=(k == 0), stop=(k == KH - 1),
            )
        srt = work.tile([1, TL], f32, tag="srt", bufs=1)
        nc.scalar.activation(srt, ms, AF.Sqrt, bias=eps_sb[:], scale=1.0 / KV_RANK)
        rinv = work.tile([1, TL], dt.float32r, tag="rinv", bufs=1)
        with nc.allow_low_precision(reason="rsqrt scale rounded to f32r for broadcast matmul"):
            nc.vector.reciprocal(rinv, srt)
        # k rope rotate (local quarter, local cos/sin)
        kr_raw = work.tile([64, TL], f32, tag="kr_raw", bufs=1)
        nc.vector.tensor_copy(kr_raw, krp)
        kr_sh = work.tile([64, TL], f32, tag="kr_sh", bufs=1)
        nc.gpsimd.dma_start(out=kr_sh[0:32, :], in_=kr_raw[32:64, :])
        nc.gpsimd.dma_start(out=kr_sh[32:64, :], in_=kr_raw[0:32, :])
        kt1 = work.tile([64, TL], f32, tag="kt1", bufs=1)
        kt2 = work.tile([64, TL], f32, tag="kt2", bufs=1)
        nc.vector.tensor_mul(kt1, kr_raw, cosl_sb[0:64, :])
        nc.vector.tensor_mul(kt2, kr_sh, sinl_sb[0:64, :])
        krl = work.tile([64, TL], bf16, tag="krl", bufs=1)
        nc.vector.tensor_sub(krl[0:32, :], kt1[0:32, :], kt2[0:32, :])
        nc.vector.tensor_add(krl[32:64, :], kt1[32:64, :], kt2[32:64, :])
        # kvcn = kvc * rsqrt(ms)
        rbc_ps = bc_ps.tile([128, TL], f32, tag="bc")
        nc.tensor.matmul(rbc_ps, onesr_sb[:], rinv[:], start=True, stop=True)
        rbc = work.tile([128, TL], f32, tag="rbc", bufs=1)
        nc.vector.tensor_copy(rbc, rbc_ps)
        kvcn = work.tile([128, RKV, TL], bf16, tag="kvcn", bufs=1)
        for j in range(RKV):
            nc.vector.tensor_mul(kvcn[:, j, :], kvc_bf[:, j, :], rbc)
        for j in range(RKV):
            nc.gpsimd.dma_start(out=gin_kv[:, ds(j * TL, TL)], in_=kvcn[:, j, :])
        nc.gpsimd.dma_start(out=gin_kv[0:64, ds(RKV * TL, TH)], in_=krl[:, 0:TH])
        nc.gpsimd.dma_start(out=gin_kv[64:128, ds(RKV * TL, TH)], in_=krl[:, TH:TL])
        nc.gpsimd.collective_compute(
            "AllGather", mybir.AluOpType.bypass, replica_groups=GROUPS,
            ins=[gin_kv.opt()], outs=[gout_kv.opt()],
        )

        # ---- q down (local quarter); pack token halves for the two gathers ----
        for m in range(RQ):
            ps = mm_ps.tile([128, TL], f32, tag="mm")
            for k in range(KH):
                nc.tensor.matmul(
                    ps, wqd_sb[:, k, ds(m * 128, 128)], xt[:, k, :],
                    start=(k == 0), stop=(k == KH - 1),
                )
            qs = work.tile([128, TL], bf16, tag="qs", bufs=3)
            nc.vector.tensor_copy(qs, ps)
            nc.gpsimd.dma_start(out=gin_qa[:, ds(m * TH, TH)], in_=qs[:, 0:TH])
            nc.gpsimd.dma_start(out=gin_qb[:, ds(m * TH, TH)], in_=qs[:, TH:TL])
        nc.gpsimd.collective_compute(
            "AllGather", mybir.AluOpType.bypass, replica_groups=GROUPS,
            ins=[gin_qa.opt()], outs=[gout_qa.opt()],
        )
        nc.gpsimd.collective_compute(
            "AllGather", mybir.AluOpType.bypass, replica_groups=GROUPS,
            ins=[gin_qb.opt()], outs=[gout_qb.opt()],
        )

        workA0.release()
        w0.release()

        # ---- SBUF-resident attention operands (live through phases B/C) ----
        res = tc.alloc_tile_pool(name="res", bufs=1)
        kn_sb = res.tile([128, NH, S], bf16, name="kn_sb")
        v_sb = res.tile([128, S // 128, NH * V_DIM], bf16, name="v_sb")
        qn_sb = res.tile([128, NH, S], bf16, name="qn_sb")
        qr_sb = res.tile([64, NH, S], bf16, name="qr_sb")
        at_sb = res.tile([128, NH, S], bf16, name="at_sb")
        krope_sb = res.tile([64, S], bf16, name="krope_sb")
        # sync queue: the gpsimd queue is occupied by the collective chain here
        mask_sb = res.tile([128, 1024], bf16, name="mask_sb")
        nc.sync.dma_start(out=mask_sb[:], in_=mask_ap[:])
        cos_sb = res.tile([128, S], bf16, name="cos_sb")
        nc.sync.dma_start(out=cos_sb[:], in_=cos_ap[:])
        sin_sb = res.tile([128, S], bf16, name="sin_sb")
        nc.sync.dma_start(out=sin_sb[:], in_=sin_ap[:])
        wkvuk_sb = res.tile([128, RKV, NH * D_NOPE], bf16, name="wkvuk_sb")
        for j in range(RKV):
            nc.sync.dma_start(out=wkvuk_sb[:, j, :], in_=wkvuk_ap[ds(j * 128, 128), :])
        wkvuv_sb = res.tile([128, RKV, NH * V_DIM], bf16, name="wkvuv_sb")
        for j in range(RKV):
            nc.sync.dma_start(out=wkvuv_sb[:, j, :], in_=wkvuv_ap[ds(j * 128, 128), :])

        workA1 = tc.alloc_tile_pool(name="workA1", bufs=2)
        work = workA1

        # ================= phase A1: kv up-projections per chunk =================
        for c in range(NT):
            csl = ds(c * T, T)
            kvg = work.tile([128, RKV * T], bf16, tag="kvg", bufs=2)
            nc.sync.dma_start(out=kvg, in_=gout_kv[c, :, ds(0, RKV * TL)])
            nc.sync.dma_start(
                out=krope_sb[:, ds(c * T, TH)], in_=gout_kv[c, 0:64, ds(RKV * TL, TH)]
            )
            nc.sync.dma_start(
                out=krope_sb[:, ds(c * T + TH, TH)], in_=gout_kv[c, 64:128, ds(RKV * TL, TH)]
            )
            for m in range(NH):
                ps = mm_ps.tile([128, T], f32, tag="mm")
                for j in range(RKV):
                    nc.tensor.matmul(
                        ps, wkvuk_sb[:, j, ds(m * 128, 128)], kvg[:, ds(j * T, T)],
                        start=(j == 0), stop=(j == RKV - 1),
                    )
                nc.vector.tensor_copy(kn_sb[:, m, csl], ps)
            for s2 in range(T // 128):
                ps = mm_ps.tile([128, NH * V_DIM], f32, tag="mm")
                for j in range(RKV):
                    nc.tensor.matmul(
                        ps, kvg[:, ds(j * T + s2 * 128, 128)], wkvuv_sb[:, j, :],
                        start=(j == 0), stop=(j == RKV - 1),
                    )
                nc.vector.tensor_copy(v_sb[:, c * (T // 128) + s2, :], ps)

        # ======== phase A2: q up-projections per gathered token half-chunk ========
        for half in (0, 1):
            for c in range(NT):
                tsl = ds(c * T + half * TH, TH)
                qlg = work.tile([128, RQ * TH], bf16, tag="qlg", bufs=2)
                if half == 0:
                    nc.sync.dma_start(out=qlg, in_=gout_qa[c, :, :])
                else:
                    nc.sync.dma_start(out=qlg, in_=gout_qb[c, :, :])
                for m in range(NH):
                    ps = mm_ps.tile([128, TH], f32, tag="mm")
                    for r in range(RQ):
                        nc.tensor.matmul(
                            ps, wqu_sb[:, r, ds(m * 128, 128)], qlg[:, ds(r * TH, TH)],
                            start=(r == 0), stop=(r == RQ - 1),
                        )
                    nc.vector.tensor_copy(qn_sb[:, m, tsl], ps)
                ps1 = mm_ps.tile([128, TH], f32, tag="mm")
                for r in range(RQ):
                    nc.tensor.matmul(
                        ps1, wqu_sb[:, r, ds(NH * D_NOPE, 128)], qlg[:, ds(r * TH, TH)],
                        start=(r == 0), stop=(r == RQ - 1),
                    )
                ps2 = mm_ps.tile([128, TH], f32, tag="mm")
                for r in range(RQ):
                    nc.tensor.matmul(
                        ps2, wqu_sb[:, r, ds(NH * D_NOPE + 128, 128)], qlg[:, ds(r * TH, TH)],
                        start=(r == 0), stop=(r == RQ - 1),
                    )
                qa = work.tile([128, TH], f32, tag="qa", bufs=1)
                qb = work.tile([128, TH], f32, tag="qb", bufs=1)
                nc.vector.tensor_mul(qa, ps1, cos_sb[:, tsl])
                nc.vector.tensor_mul(qb, ps2, sin_sb[:, tsl])
                y1 = work.tile([128, TH], bf16, tag="y1", bufs=2)
                nc.vector.tensor_sub(y1, qa, qb)
                qa2 = work.tile([128, TH], f32, tag="qa", bufs=1)
                qb2 = work.tile([128, TH], f32, tag="qb", bufs=1)
                nc.vector.tensor_mul(qa2, ps2, cos_sb[:, tsl])
                nc.vector.tensor_mul(qb2, ps1, sin_sb[:, tsl])
                y2 = work.tile([128, TH], bf16, tag="y2", bufs=2)
                nc.vector.tensor_add(y2, qa2, qb2)
                for h in range(NH):
                    nc.gpsimd.dma_start(out=qr_sb[0:32, h, tsl], in_=y1[ds(32 * h, 32), :])
                    nc.gpsimd.dma_start(out=qr_sb[32:64, h, tsl], in_=y2[ds(32 * h, 32), :])
        workA1.release()

        # w_out preload: sync queue is idle during phase B
        wo_pool = tc.alloc_tile_pool(name="wo_pool", bufs=1)
        wo_ts = []
        for n in range(HID // 512):
            wo_t = wo_pool.tile([128, NH, 512], bf16, name=f"wo{n}")
            for f in range(NH):
                nc.sync.dma_start(
                    out=wo_t[:, f, :], in_=wout_ap[ds(f * 128, 128), ds(n * 512, 512)]
                )
            wo_ts.append(wo_t)

        # ================= phase B: attention =================
        workB = tc.alloc_tile_pool(name="workB", bufs=2)
        work = workB

        def drain_norm(st):
            # deferred normalize: by now rec (DVE) has long finished
            h_, qsl_, pv_, rec_ = st
            rb2_ps = bc_ps.tile([128, QC], f32, tag="bc")
            nc.tensor.matmul(rb2_ps, onesr_sb[:], rec_[:], start=True, stop=True)
            rbs = work.tile([128, QC], f32, tag="rbs", bufs=2)
            nc.vector.tensor_copy(rbs, rb2_ps)
            nc.vector.tensor_mul(at_sb[:, h_, qsl_], pv_, rbs)

        def emit_out(t16):
            # one 128-token out-projection block; fills PE slivers in B and
            # spreads the output DMA across the whole attention phase
            for n in range(HID // 512):
                psC = mm_ps.tile([128, 512], f32, tag="mm")
                for fc in range(NH):
                    nc.tensor.matmul(
                        psC, at_sb[:, fc, ds(t16 * 128, 128)], wo_ts[n][:, fc, :],
                        start=(fc == 0), stop=(fc == NH - 1),
                    )
                o_t = work.tile([128, 512], bf16, tag="ot", bufs=3)
                nc.vector.tensor_copy(o_t, psC)
                nc.sync.dma_start(
                    out=out_ap[ds(t16 * 128, 128), ds(n * 512, 512)], in_=o_t
                )

        norm_pend = []
        for qc in range(NQC):
            qsl = ds(qc * QC, QC)
            nkc = 4 * qc + 4
            for h in range(NH):
                pv = pv_ps.tile([128, QC], f32, tag="pv")
                den = sm_ps.tile([1, QC], f32, tag="rowps", bufs=1)
                pend = []

                def drain_block(blk, final):
                    # den x block then pv x block: same-bank matmuls adjacent
                    for bi, (pkc, pE) in enumerate(blk):
                        nc.tensor.matmul(
                            den, onesc_sb[:], pE, start=(pkc == 0),
                            stop=(final and bi == len(blk) - 1),
                        )
                    for bi, (pkc, pE) in enumerate(blk):
                        nc.tensor.matmul(
                            pv, v_sb[:, pkc, ds(h * V_DIM, V_DIM)], pE,
                            start=(pkc == 0), stop=(final and bi == len(blk) - 1),
                        )

                for kc in range(nkc):
                    sps = mm_ps.tile([128, QC], f32, tag="mm")
                    nc.tensor.matmul(
                        sps, kn_sb[:, h, ds(kc * 128, 128)], qn_sb[:, h, qsl],
                        start=True, stop=False,
                    )
                    nc.tensor.matmul(
                        sps, krope_sb[:, ds(kc * 128, 128)], qr_sb[:, h, qsl],
                        start=False, stop=True,
                    )
                    E = work.tile([128, QC], bf16, tag="E", bufs=12)
                    nc.scalar.activation(E, sps, AF.Exp, scale=SCALE)
                    dm = kc - 4 * qc
                    if dm >= 0:
                        nc.gpsimd.tensor_mul(E, E, mask_sb[:, ds(512 - 128 * dm, 512)])
                    pend.append((kc, E))
                    if len(pend) == 8:  # drain oldest block one block behind
                        drain_block(pend[:4], final=False)
                        del pend[:4]
                drain_block(pend, final=True)
                pend = []
                rec = work.tile([1, QC], dt.float32r, tag="rec", bufs=2)
                with nc.allow_low_precision(reason="softmax denom rounded to f32r for broadcast matmul"):
                    nc.vector.reciprocal(rec, den)
                norm_pend.append((h, qsl, pv, rec))
                if len(norm_pend) > 1:
                    drain_norm(norm_pend.pop(0))
        while norm_pend:
            drain_norm(norm_pend.pop(0))

        # ================= phase C: out-projection =================
        for t16 in range(NQC * 4):
            emit_out(t16)
        workB.release()
        wo_pool.release()
        res.release()
        wAp.release()

    nc.compile()
    return nc


def get_nc():
    if "nc" not in _CACHE:
        _CACHE["nc"] = build_nc()
    return _CACHE["nc"]


def host_inputs(x, w_q_down, w_q_up, w_kv_down, kv_norm_w, w_kv_up, w_out):
    """Build the 8 per-core input shards (host-side prep, numpy only)."""
    bf = ml_dtypes.bfloat16
    x = np.asarray(x, np.float32)
    inv = 1.0 / THETA ** (np.arange(0, D_ROPE, 2, dtype=np.float64) / D_ROPE)
    ang = np.arange(S, dtype=np.float64)[:, None] * inv[None, :]      # (S, 32)
    cosq = np.ascontiguousarray(np.tile(np.cos(ang).T, (4, 1))).astype(bf)  # (128, S)
    sinq = np.ascontiguousarray(np.tile(np.sin(ang).T, (4, 1))).astype(bf)
    maskt = (
        np.arange(1024)[None, :] >= (np.arange(128)[:, None] + 512)
    ).astype(bf)
    ones_col = np.ones((128, 1), bf)
    ones_row = np.ones((1, 128), np.float32)
    wkv_eff = np.asarray(w_kv_up, np.float32) * np.asarray(kv_norm_w, np.float32)[:, None]

    xT_bf = [np.ascontiguousarray(x[b].T).astype(bf) for b in range(B)]
    wqd_bf = np.asarray(w_q_down, np.float32).astype(bf)
    wkvd_bf = np.asarray(w_kv_down, np.float32).astype(bf)
    wqu_f = np.asarray(w_q_up, np.float32)
    wout_f = np.asarray(w_out, np.float32)

    in_maps = []
    for ci in range(NCORES):
        b, hg = divmod(ci, 4)
        heads = list(range(NH * hg, NH * hg + NH))
        qu_cols = (
            [h * HEAD_DIM + j for h in heads for j in range(D_NOPE)]
            + [h * HEAD_DIM + D_NOPE + j for h in heads for j in range(32)]
            + [h * HEAD_DIM + D_NOPE + 32 + j for h in heads for j in range(32)]
        )
        kn_cols = [h * (D_NOPE + V_DIM) + j for h in heads for j in range(D_NOPE)]
        v_cols = [h * (D_NOPE + V_DIM) + D_NOPE + j for h in heads for j in range(V_DIM)]
        in_maps.append(
            {
                "x": np.ascontiguousarray(xT_bf[b][:, 512 * hg : 512 * (hg + 1)]),
                "cosl": np.ascontiguousarray(cosq[:, 512 * hg : 512 * (hg + 1)]),
                "sinl": np.ascontiguousarray(sinq[:, 512 * hg : 512 * (hg + 1)]),
                "wqd": wqd_bf,
                "wqu": np.ascontiguousarray(wqu_f[:, qu_cols]).astype(bf),
                "wkvd": wkvd_bf,
                "wkvuk": np.ascontiguousarray(wkv_eff[:, kn_cols]).astype(bf),
                "wkvuv": np.ascontiguousarray(wkv_eff[:, v_cols]).astype(bf),
                "wout": np.ascontiguousarray(
                    wout_f[NH * V_DIM * hg : NH * V_DIM * (hg + 1), :]
                ).astype(bf),
                "cosq": cosq,
                "sinq": sinq,
                "maskt": maskt,
                "ones_col": ones_col,
                "ones_row": ones_row,
            }
        )
    return in_maps


def run(inputs, trace=False, trace_cores=None):
    from concourse.bass_utils import run_bass_kernel_spmd

    nc = get_nc()
    in_maps = host_inputs(**inputs)
    res = run_bass_kernel_spmd(
        nc,
        in_maps,
        core_ids=list(range(NCORES)),
        trace=trace,
        trace_cores=trace_cores,
    )
    out = np.zeros((B, S, HID), np.float32)
    for ci in range(NCORES):
        out[ci // 4] += np.asarray(res.results[ci]["out"], dtype=np.float32)
    return out, res


def kernel(**inputs):
    out, _ = run(inputs, trace=False)
    return out


# revision 54
# speedup vs baseline: 1.1383x; 1.1383x over previous
"""MLA attention kernel for Trainium2 — 8-core tensor-parallel (self-contained).

Sharding: data-parallel over batch (2) x tensor-parallel over head groups
(4 groups of 4 heads) = 8 cores, SPMD (one NEFF, per-core input shards).
Core ci: batch ci//4, heads [4*(ci%4), 4*(ci%4)+4).

Per-core dataflow (everything feature-major "transposed" so the PE never
needs an on-chip transpose):
  x^T quarter + weights streamed in up front (wkvd/xt interleaved first)
  kv^T = wkvd.T @ x^T (rmsnorm sum via ones-matmul, scale broadcast via K=1
    matmul); normalized latents AllGather'd in bf16 (dispatched early so the
    wire hides under the q path)
  q_lat^T = wqd.T @ x^T, AllGather'd bf16 in two token halves so q-up can
    start on the first half while the second is still on the wire
  kv-up / q-up consume the gathered latents; k_nope^T / v / q^T / q_rope^T
    live in SBUF-RESIDENT tiles -- phase B issues ZERO DMA.
  scores^T[k,q] = k^T.T @ q^T -> exp (ACT, scale folded) -> causal mask (GpSimd)
  denom[1,q] = ones.T @ E ; out_h^T[v,q] = v.T @ E, drained in 4-kc blocks so
    same-PSUM-bank matmuls stay adjacent (short accumulation groups pay a
    ~150ns group-start penalty on the PE)
  normalize via K=1 broadcast matmul of 1/denom, deferred one iteration
  out[t, hid] = attn^T.T @ w_out (token-major, bf16 contiguous writes)
Host: sums the 4 bf16 partial outputs per batch in fp32.
"""

import math

import numpy as np
import ml_dtypes

# ---- problem constants (from the reference model) ----
B, S, HID = 2, 2048, 2048
H, D_NOPE, D_ROPE, V_DIM = 16, 128, 64, 128
KV_RANK, Q_RANK = 512, 1536
HEAD_DIM = D_NOPE + D_ROPE
THETA, EPS = 10000.0, 1e-6
NCORES = 8
NH = 4                    # heads per core
T = 512                   # phase-A token chunk
NT = S // T
QC = 512                  # attention query chunk
NQC = S // QC
KH = HID // 128           # 16 k-chunks over HID
RQ = Q_RANK // 128        # 12 chunks over q rank
RKV = KV_RANK // 128      # 4 chunks over kv rank
SCALE = 1.0 / math.sqrt(HEAD_DIM)

_CACHE = {}


def build_nc():
    """Build the Bass/Tile program (one NeuronCore, run SPMD on 8)."""
    from contextlib import ExitStack

    import concourse.mybir as mybir
    import concourse.tile as tile
    from concourse import bacc
    from concourse.bass import ds

    dt = mybir.dt
    AF = mybir.ActivationFunctionType
    bf16 = dt.bfloat16
    f32 = dt.float32

    nc = bacc.Bacc(
        "TRN2",
        target_bir_lowering=False,
        debug=False,
        enable_asserts=False,
        num_devices=NCORES,
    )

    # ---- I/O ----
    x_ap = nc.dram_tensor("x", [HID, S // 4], bf16, kind="ExternalInput").ap()
    wqd_ap = nc.dram_tensor("wqd", [HID, Q_RANK], bf16, kind="ExternalInput").ap()
    wqu_ap = nc.dram_tensor("wqu", [Q_RANK, NH * HEAD_DIM], bf16, kind="ExternalInput").ap()
    cos_ap = nc.dram_tensor("cosq", [128, S], bf16, kind="ExternalInput").ap()
    sin_ap = nc.dram_tensor("sinq", [128, S], bf16, kind="ExternalInput").ap()
    wkvd_ap = nc.dram_tensor("wkvd", [HID, KV_RANK + D_ROPE], bf16, kind="ExternalInput").ap()
    wkvuk_ap = nc.dram_tensor("wkvuk", [KV_RANK, NH * D_NOPE], bf16, kind="ExternalInput").ap()
    wkvuv_ap = nc.dram_tensor("wkvuv", [KV_RANK, NH * V_DIM], bf16, kind="ExternalInput").ap()
    wout_ap = nc.dram_tensor("wout", [NH * V_DIM, HID], bf16, kind="ExternalInput").ap()
    mask_ap = nc.dram_tensor("maskt", [128, 1024], bf16, kind="ExternalInput").ap()
    onesc_ap = nc.dram_tensor("ones_col", [128, 1], bf16, kind="ExternalInput").ap()
    onesr_ap = nc.dram_tensor("ones_row", [1, 128], f32, kind="ExternalInput").ap()
    cosl_ap = nc.dram_tensor("cosl", [128, S // 4], bf16, kind="ExternalInput").ap()
    sinl_ap = nc.dram_tensor("sinl", [128, S // 4], bf16, kind="ExternalInput").ap()
    out_ap = nc.dram_tensor("out", [S, HID], bf16, kind="ExternalOutput").ap()

    with tile.TileContext(nc) as tc, ExitStack() as ctx:
        const = ctx.enter_context(tc.tile_pool(name="const", bufs=1))
        dram = ctx.enter_context(tc.tile_pool(name="dram", bufs=1, space="DRAM"))
        mm_ps = ctx.enter_context(tc.tile_pool(name="mm_ps", bufs=4, space="PSUM"))
        pv_ps = ctx.enter_context(tc.tile_pool(name="pv_ps", bufs=2, space="PSUM"))
        sm_ps = ctx.enter_context(tc.tile_pool(name="sm_ps", bufs=1, space="PSUM"))
        bc_ps = ctx.enter_context(tc.tile_pool(name="bc_ps", bufs=1, space="PSUM"))

        TL = S // 4  # local token quarter
        TH = TL // 2  # gather token half

        # ---- pools by lifetime (releases must nest LIFO) ----
        wAp = tc.alloc_tile_pool(name="wAp", bufs=1)  # wqu slice (small, kept)
        w0 = tc.alloc_tile_pool(name="w0", bufs=1)    # wkvd, xt, wqd: dead after A0

        # DMA issue order is consumption order: wkvd/xt interleaved (kv down
        # paces with these), then wqd, then the q-up slice.
        wkvd_sb = w0.tile([128, KH, KV_RANK + D_ROPE], bf16, name="wkvd_sb")
        xt = w0.tile([128, KH, TL], bf16, name="xt_sb")
        for k in range(KH):
            nc.sync.dma_start(out=wkvd_sb[:, k, :], in_=wkvd_ap[ds(k * 128, 128), :])
            nc.sync.dma_start(out=xt[:, k, :], in_=x_ap[ds(k * 128, 128), :])
        wqd_sb = w0.tile([128, KH, Q_RANK], bf16, name="wqd_sb")
        for k in range(KH):
            nc.sync.dma_start(out=wqd_sb[:, k, :], in_=wqd_ap[ds(k * 128, 128), :])
        wqu_sb = wAp.tile([128, RQ, NH * HEAD_DIM], bf16, name="wqu_sb")
        for r in range(RQ):
            nc.sync.dma_start(out=wqu_sb[:, r, :], in_=wqu_ap[ds(r * 128, 128), :])

        # small constants on the gpsimd queue (doesn't delay the weight stream)
        cosl_sb = const.tile([128, TL], bf16, name="cosl_sb")
        nc.gpsimd.dma_start(out=cosl_sb[:], in_=cosl_ap[:])
        sinl_sb = const.tile([128, TL], bf16, name="sinl_sb")
        nc.gpsimd.dma_start(out=sinl_sb[:], in_=sinl_ap[:])
        onesc_sb = const.tile([128, 1], bf16, name="onesc_sb")
        nc.gpsimd.dma_start(out=onesc_sb[:], in_=onesc_ap[:])
        onesr_f32 = const.tile([1, 128], f32, name="onesr_f32")
        nc.gpsimd.dma_start(out=onesr_f32[:], in_=onesr_ap[:])
        onesr_sb = const.tile([1, 128], dt.float32r, name="onesr_sb")
        with nc.allow_low_precision(reason="exact ones rounded to f32r"):
            nc.vector.tensor_copy(onesr_sb[:], onesr_f32[:])
        eps_sb = const.tile([1, 1], f32, name="eps_sb")
        nc.gpsimd.memset(eps_sb[:], EPS)

        # DRAM collective bounce buffers (q latent gathered in token halves
        # so its wire pipelines into half-granular q-up work)
        # Payloads are partition-major [128, free] so each consumer quarter
        # unpacks with ONE wide DMA instead of one per rank-chunk.  k-rope is
        # folded into [128, TL/2] (token halves stacked) — zero wire waste.
        gin_kv = dram.tile([128, RKV * TL + TH], bf16, name="gin_kv")
        gout_kv = dram.tile([4, 128, RKV * TL + TH], bf16, name="gout_kv")
        gin_qa = dram.tile([128, RQ * TH], bf16, name="gin_qa")
        gout_qa = dram.tile([4, 128, RQ * TH], bf16, name="gout_qa")
        gin_qb = dram.tile([128, RQ * TH], bf16, name="gin_qb")
        gout_qb = dram.tile([4, 128, RQ * TH], bf16, name="gout_qb")
        GROUPS = [[0, 1, 2, 3], [4, 5, 6, 7]]

        # ================= phase A0: local down-projections =================
        workA0 = tc.alloc_tile_pool(name="workA0", bufs=2)
        work = workA0
        # ---- kv down (local quarter) ----
        kvc_bf = work.tile([128, RKV, TL], bf16, tag="kvc", bufs=1)
        sq_bf = work.tile([128, RKV, TL], bf16, tag="sq", bufs=1)
        for j in range(RKV):
            ps = mm_ps.tile([128, TL], f32, tag="mm")
            for k in range(KH):
                nc.tensor.matmul(
                    ps, wkvd_sb[:, k, ds(j * 128, 128)], xt[:, k, :],
                    start=(k == 0), stop=(k == KH - 1),
                )
            nc.scalar.activation(sq_bf[:, j, :], ps, AF.Square)
            nc.vector.tensor_copy(kvc_bf[:, j, :], ps)
        ms = sm_ps.tile([1, TL], f32, tag="rowps", bufs=1)
        for j in range(RKV):
            nc.tensor.matmul(
                ms, onesc_sb[:], sq_bf[:, j, :],
                start=(j == 0), stop=(j == RKV - 1),
            )
        krp = mm_ps.tile([64, TL], f32, tag="mm")
        for k in range(KH):
            nc.tensor.matmul(
                krp, wkvd_sb[:, k, ds(KV_RANK, D_ROPE)], xt[:, k, :],
                start=(k == 0), stop=(k == KH - 1),
            )
        srt = work.tile([1, TL], f32, tag="srt", bufs=1)
        nc.scalar.activation(srt, ms, AF.Sqrt, bias=eps_sb[:], scale=1.0 / KV_RANK)
        rinv = work.tile([1, TL], dt.float32r, tag="rinv", bufs=1)
        with nc.allow_low_precision(reason="rsqrt scale rounded to f32r for broadcast matmul"):
            nc.vector.reciprocal(rinv, srt)
        # k rope rotate (local quarter, local cos/sin)
        kr_raw = work.tile([64, TL], f32, tag="kr_raw", bufs=1)
        nc.vector.tensor_copy(kr_raw, krp)
        kr_sh = work.tile([64, TL], f32, tag="kr_sh", bufs=1)
        nc.gpsimd.dma_start(out=kr_sh[0:32, :], in_=kr_raw[32:64, :])
        nc.gpsimd.dma_start(out=kr_sh[32:64, :], in_=kr_raw[0:32, :])
        kt1 = work.tile([64, TL], f32, tag="kt1", bufs=1)
        kt2 = work.tile([64, TL], f32, tag="kt2", bufs=1)
        nc.vector.tensor_mul(kt1, kr_raw, cosl_sb[0:64, :])
        nc.vector.tensor_mul(kt2, kr_sh, sinl_sb[0:64, :])
        krl = work.tile([64, TL], bf16, tag="krl", bufs=1)
        nc.vector.tensor_sub(krl[0:32, :], kt1[0:32, :], kt2[0:32, :])
        nc.vector.tensor_add(krl[32:64, :], kt1[32:64, :], kt2[32:64, :])
        # kvcn = kvc * rsqrt(ms)
        rbc_ps = bc_ps.tile([128, TL], f32, tag="bc")
        nc.tensor.matmul(rbc_ps, onesr_sb[:], rinv[:], start=True, stop=True)
        rbc = work.tile([128, TL], f32, tag="rbc", bufs=1)
        nc.vector.tensor_copy(rbc, rbc_ps)
        kvcn = work.tile([128, RKV, TL], bf16, tag="kvcn", bufs=1)
        for j in range(RKV):
            nc.vector.tensor_mul(kvcn[:, j, :], kvc_bf[:, j, :], rbc)
        for j in range(RKV):
            nc.gpsimd.dma_start(out=gin_kv[:, ds(j * TL, TL)], in_=kvcn[:, j, :])
        nc.gpsimd.dma_start(out=gin_kv[0:64, ds(RKV * TL, TH)], in_=krl[:, 0:TH])
        nc.gpsimd.dma_start(out=gin_kv[64:128, ds(RKV * TL, TH)], in_=krl[:, TH:TL])
        nc.gpsimd.collective_compute(
            "AllGather", mybir.AluOpType.bypass, replica_groups=GROUPS,
            ins=[gin_kv.opt()], outs=[gout_kv.opt()],
        )

        # ---- q down (local quarter); pack token halves for the two gathers ----
        for m in range(RQ):
            ps = mm_ps.tile([128, TL], f32, tag="mm")
            for k in range(KH):
                nc.tensor.matmul(
                    ps, wqd_sb[:, k, ds(m * 128, 128)], xt[:, k, :],
                    start=(k == 0), stop=(k == KH - 1),
                )
            qs = work.tile([128, TL], bf16, tag="qs", bufs=3)
            nc.vector.tensor_copy(qs, ps)
            nc.gpsimd.dma_start(out=gin_qa[:, ds(m * TH, TH)], in_=qs[:, 0:TH])
            nc.gpsimd.dma_start(out=gin_qb[:, ds(m * TH, TH)], in_=qs[:, TH:TL])
        nc.gpsimd.collective_compute(
            "AllGather", mybir.AluOpType.bypass, replica_groups=GROUPS,
            ins=[gin_qa.opt()], outs=[gout_qa.opt()],
        )
        nc.gpsimd.collective_compute(
            "AllGather", mybir.AluOpType.bypass, replica_groups=GROUPS,
            ins=[gin_qb.opt()], outs=[gout_qb.opt()],
        )

        workA0.release()
        w0.release()

        # ---- SBUF-resident attention operands (live through phases B/C) ----
        res = tc.alloc_tile_pool(name="res", bufs=1)
        kn_sb = res.tile([128, NH, S], bf16, name="kn_sb")
        v_sb = res.tile([128, S // 128, NH * V_DIM], bf16, name="v_sb")
        qn_sb = res.tile([128, NH, S], bf16, name="qn_sb")
        qr_sb = res.tile([64, NH, S], bf16, name="qr_sb")
        at_sb = res.tile([128, NH, S], bf16, name="at_sb")
        krope_sb = res.tile([64, S], bf16, name="krope_sb")
        # sync queue: the gpsimd queue is occupied by the collective chain here
        mask_sb = res.tile([128, 1024], bf16, name="mask_sb")
        nc.sync.dma_start(out=mask_sb[:], in_=mask_ap[:])
        cos_sb = res.tile([128, S], bf16, name="cos_sb")
        nc.sync.dma_start(out=cos_sb[:], in_=cos_ap[:])
        sin_sb = res.tile([128, S], bf16, name="sin_sb")
        nc.sync.dma_start(out=sin_sb[:], in_=sin_ap[:])
        wkvuk_sb = res.tile([128, RKV, NH * D_NOPE], bf16, name="wkvuk_sb")
        for j in range(RKV):
            nc.sync.dma_start(out=wkvuk_sb[:, j, :], in_=wkvuk_ap[ds(j * 128, 128), :])
        wkvuv_sb = res.tile([128, RKV, NH * V_DIM], bf16, name="wkvuv_sb")
        for j in range(RKV):
            nc.sync.dma_start(out=wkvuv_sb[:, j, :], in_=wkvuv_ap[ds(j * 128, 128), :])

        workA1 = tc.alloc_tile_pool(name="workA1", bufs=2)
        work = workA1

        # ================= phase A1: kv up-projections per chunk =================
        for c in range(NT):
            csl = ds(c * T, T)
            kvg = work.tile([128, RKV * T], bf16, tag="kvg", bufs=2)
            nc.sync.dma_start(out=kvg, in_=gout_kv[c, :, ds(0, RKV * TL)])
            nc.sync.dma_start(
                out=krope_sb[:, ds(c * T, TH)], in_=gout_kv[c, 0:64, ds(RKV * TL, TH)]
            )
            nc.sync.dma_start(
                out=krope_sb[:, ds(c * T + TH, TH)], in_=gout_kv[c, 64:128, ds(RKV * TL, TH)]
            )
            for m in range(NH):
                ps = mm_ps.tile([128, T], f32, tag="mm")
                for j in range(RKV):
                    nc.tensor.matmul(
                        ps, wkvuk_sb[:, j, ds(m * 128, 128)], kvg[:, ds(j * T, T)],
                        start=(j == 0), stop=(j == RKV - 1),
                    )
                nc.vector.tensor_copy(kn_sb[:, m, csl], ps)
            for s2 in range(T // 128):
                ps = mm_ps.tile([128, NH * V_DIM], f32, tag="mm")
                for j in range(RKV):
                    nc.tensor.matmul(
                        ps, kvg[:, ds(j * T + s2 * 128, 128)], wkvuv_sb[:, j, :],
                        start=(j == 0), stop=(j == RKV - 1),
                    )
                nc.vector.tensor_copy(v_sb[:, c * (T // 128) + s2, :], ps)

        # ======== phase A2: q up-projections per gathered token half-chunk ========
        for half in (0, 1):
            for c in range(NT):
                tsl = ds(c * T + half * TH, TH)
                qlg = work.tile([128, RQ * TH], bf16, tag="qlg", bufs=2)
                if half == 0:
                    nc.sync.dma_start(out=qlg, in_=gout_qa[c, :, :])
                else:
                    nc.sync.dma_start(out=qlg, in_=gout_qb[c, :, :])
                for m in range(NH):
                    ps = mm_ps.tile([128, TH], f32, tag="mm")
                    for r in range(RQ):
                        nc.tensor.matmul(
                            ps, wqu_sb[:, r, ds(m * 128, 128)], qlg[:, ds(r * TH, TH)],
                            start=(r == 0), stop=(r == RQ - 1),
                        )
                    nc.vector.tensor_copy(qn_sb[:, m, tsl], ps)
                ps1 = mm_ps.tile([128, TH], f32, tag="mm")
                for r in range(RQ):
                    nc.tensor.matmul(
                        ps1, wqu_sb[:, r, ds(NH * D_NOPE, 128)], qlg[:, ds(r * TH, TH)],
                        start=(r == 0), stop=(r == RQ - 1),
                    )
                ps2 = mm_ps.tile([128, TH], f32, tag="mm")
                for r in range(RQ):
                    nc.tensor.matmul(
                        ps2, wqu_sb[:, r, ds(NH * D_NOPE + 128, 128)], qlg[:, ds(r * TH, TH)],
                        start=(r == 0), stop=(r == RQ - 1),
                    )
                qa = work.tile([128, TH], f32, tag="qa", bufs=1)
                qb = work.tile([128, TH], f32, tag="qb", bufs=1)
                nc.vector.tensor_mul(qa, ps1, cos_sb[:, tsl])
                nc.vector.tensor_mul(qb, ps2, sin_sb[:, tsl])
                y1 = work.tile([128, TH], bf16, tag="y1", bufs=2)
                nc.vector.tensor_sub(y1, qa, qb)
                qa2 = work.tile([128, TH], f32, tag="qa", bufs=1)
                qb2 = work.tile([128, TH], f32, tag="qb", bufs=1)
                nc.vector.tensor_mul(qa2, ps2, cos_sb[:, tsl])
                nc.vector.tensor_mul(qb2, ps1, sin_sb[:, tsl])
                y2 = work.tile([128, TH], bf16, tag="y2", bufs=2)
                nc.vector.tensor_add(y2, qa2, qb2)
                for h in range(NH):
                    nc.gpsimd.dma_start(out=qr_sb[0:32, h, tsl], in_=y1[ds(32 * h, 32), :])
                    nc.gpsimd.dma_start(out=qr_sb[32:64, h, tsl], in_=y2[ds(32 * h, 32), :])
        workA1.release()

        # w_out preload: sync queue is idle during phase B
        wo_pool = tc.alloc_tile_pool(name="wo_pool", bufs=1)
        wo_ts = []
        for n in range(HID // 512):
            wo_t = wo_pool.tile([128, NH, 512], bf16, name=f"wo{n}")
            for f in range(NH):
                nc.sync.dma_start(
                    out=wo_t[:, f, :], in_=wout_ap[ds(f * 128, 128), ds(n * 512, 512)]
                )
            wo_ts.append(wo_t)

        # ================= phase B: attention =================
        workB = tc.alloc_tile_pool(name="workB", bufs=2)
        work = workB

        def drain_norm(st):
            # deferred normalize: by now rec (DVE) has long finished
            h_, qsl_, pv_, rec_ = st
            rb2_ps = bc_ps.tile([128, QC], f32, tag="bc")
            nc.tensor.matmul(rb2_ps, onesr_sb[:], rec_[:], start=True, stop=True)
            rbs = work.tile([128, QC], f32, tag="rbs", bufs=2)
            nc.vector.tensor_copy(rbs, rb2_ps)
            nc.vector.tensor_mul(at_sb[:, h_, qsl_], pv_, rbs)

        norm_pend = []
        for qc in range(NQC):
            qsl = ds(qc * QC, QC)
            nkc = 4 * qc + 4
            for h in range(NH):
                pv = pv_ps.tile([128, QC], f32, tag="pv")
                den = sm_ps.tile([1, QC], f32, tag="rowps", bufs=1)
                pend = []

                def drain_block(blk, final):
                    # den x block then pv x block: same-bank matmuls adjacent
                    for bi, (pkc, pE) in enumerate(blk):
                        nc.tensor.matmul(
                            den, onesc_sb[:], pE, start=(pkc == 0),
                            stop=(final and bi == len(blk) - 1),
                        )
                    for bi, (pkc, pE) in enumerate(blk):
                        nc.tensor.matmul(
                            pv, v_sb[:, pkc, ds(h * V_DIM, V_DIM)], pE,
                            start=(pkc == 0), stop=(final and bi == len(blk) - 1),
                        )

                for kc in range(nkc):
                    sps = mm_ps.tile([128, QC], f32, tag="mm")
                    nc.tensor.matmul(
                        sps, kn_sb[:, h, ds(kc * 128, 128)], qn_sb[:, h, qsl],
                        start=True, stop=False,
                    )
                    nc.tensor.matmul(
                        sps, krope_sb[:, ds(kc * 128, 128)], qr_sb[:, h, qsl],
                        start=False, stop=True,
                    )
                    E = work.tile([128, QC], bf16, tag="E", bufs=12)
                    nc.scalar.activation(E, sps, AF.Exp, scale=SCALE)
                    dm = kc - 4 * qc
                    if dm >= 0:
                        nc.gpsimd.tensor_mul(E, E, mask_sb[:, ds(512 - 128 * dm, 512)])
                    pend.append((kc, E))
                    if len(pend) == 8:  # drain oldest block one block behind
                        drain_block(pend[:4], final=False)
                        del pend[:4]
                drain_block(pend, final=True)
                pend = []
                rec = work.tile([1, QC], dt.float32r, tag="rec", bufs=2)
                with nc.allow_low_precision(reason="softmax denom rounded to f32r for broadcast matmul"):
                    nc.vector.reciprocal(rec, den)
                norm_pend.append((h, qsl, pv, rec))
                if len(norm_pend) > 1:
                    drain_norm(norm_pend.pop(0))
        while norm_pend:
            drain_norm(norm_pend.pop(0))

        # ================= phase C: out-projection =================
        # n-outer: the moving operand (wo_ts[n]) stays fixed across 16
        # consecutive psum groups — the fast LDW-overlapped matmul regime
        for n in range(HID // 512):
            for t16 in range(NQC * 4):
                psC = mm_ps.tile([128, 512], f32, tag="mm")
                for fc in range(NH):
                    nc.tensor.matmul(
                        psC, at_sb[:, fc, ds(t16 * 128, 128)], wo_ts[n][:, fc, :],
                        start=(fc == 0), stop=(fc == NH - 1),
                    )
                o_t = work.tile([128, 512], bf16, tag="ot", bufs=3)
                nc.vector.tensor_copy(o_t, psC)
                nc.sync.dma_start(
                    out=out_ap[ds(t16 * 128, 128), ds(n * 512, 512)], in_=o_t
                )
        workB.release()
        wo_pool.release()
        res.release()
        wAp.release()

    nc.compile()
    return nc


def get_nc():
    if "nc" not in _CACHE:
        _CACHE["nc"] = build_nc()
    return _CACHE["nc"]


def host_inputs(x, w_q_down, w_q_up, w_kv_down, kv_norm_w, w_kv_up, w_out):
    """Build the 8 per-core input shards (host-side prep, numpy only)."""
    bf = ml_dtypes.bfloat16
    x = np.asarray(x, np.float32)
    inv = 1.0 / THETA ** (np.arange(0, D_ROPE, 2, dtype=np.float64) / D_ROPE)
    ang = np.arange(S, dtype=np.float64)[:, None] * inv[None, :]      # (S, 32)
    cosq = np.ascontiguousarray(np.tile(np.cos(ang).T, (4, 1))).astype(bf)  # (128, S)
    sinq = np.ascontiguousarray(np.tile(np.sin(ang).T, (4, 1))).astype(bf)
    maskt = (
        np.arange(1024)[None, :] >= (np.arange(128)[:, None] + 512)
    ).astype(bf)
    ones_col = np.ones((128, 1), bf)
    ones_row = np.ones((1, 128), np.float32)
    wkv_eff = np.asarray(w_kv_up, np.float32) * np.asarray(kv_norm_w, np.float32)[:, None]

    xT_bf = [np.ascontiguousarray(x[b].T).astype(bf) for b in range(B)]
    wqd_bf = np.asarray(w_q_down, np.float32).astype(bf)
    wkvd_bf = np.asarray(w_kv_down, np.float32).astype(bf)
    wqu_f = np.asarray(w_q_up, np.float32)
    wout_f = np.asarray(w_out, np.float32)

    in_maps = []
    for ci in range(NCORES):
        b, hg = divmod(ci, 4)
        heads = list(range(NH * hg, NH * hg + NH))
        qu_cols = (
            [h * HEAD_DIM + j for h in heads for j in range(D_NOPE)]
            + [h * HEAD_DIM + D_NOPE + j for h in heads for j in range(32)]
            + [h * HEAD_DIM + D_NOPE + 32 + j for h in heads for j in range(32)]
        )
        kn_cols = [h * (D_NOPE + V_DIM) + j for h in heads for j in range(D_NOPE)]
        v_cols = [h * (D_NOPE + V_DIM) + D_NOPE + j for h in heads for j in range(V_DIM)]
        in_maps.append(
            {
                "x": np.ascontiguousarray(xT_bf[b][:, 512 * hg : 512 * (hg + 1)]),
                "cosl": np.ascontiguousarray(cosq[:, 512 * hg : 512 * (hg + 1)]),
                "sinl": np.ascontiguousarray(sinq[:, 512 * hg : 512 * (hg + 1)]),
                "wqd": wqd_bf,
                "wqu": np.ascontiguousarray(wqu_f[:, qu_cols]).astype(bf),
                "wkvd": wkvd_bf,
                "wkvuk": np.ascontiguousarray(wkv_eff[:, kn_cols]).astype(bf),
                "wkvuv": np.ascontiguousarray(wkv_eff[:, v_cols]).astype(bf),
                "wout": np.ascontiguousarray(
                    wout_f[NH * V_DIM * hg : NH * V_DIM * (hg + 1), :]
                ).astype(bf),
                "cosq": cosq,
                "sinq": sinq,
                "maskt": maskt,
                "ones_col": ones_col,
                "ones_row": ones_row,
            }
        )
    return in_maps


def run(inputs, trace=False, trace_cores=None):
    from concourse.bass_utils import run_bass_kernel_spmd

    nc = get_nc()
    in_maps = host_inputs(**inputs)
    res = run_bass_kernel_spmd(
        nc,
        in_maps,
        core_ids=list(range(NCORES)),
        trace=trace,
        trace_cores=trace_cores,
    )
    out = np.zeros((B, S, HID), np.float32)
    for ci in range(NCORES):
        out[ci // 4] += np.asarray(res.results[ci]["out"], dtype=np.float32)
    return out, res


def kernel(**inputs):
    out, _ = run(inputs, trace=False)
    return out
